# revision 6
# baseline (speedup 1.0000x reference)
"""CayleyConv forward on 8 Trainium2 NeuronCores (Bass/Tile).

Problem: x [16,16,128,128,3,3] f32, g [16,16,8,3] f32
         -> out [16,16,130,130,3,3] f32.

Math (faithful to the reference's sequential-overwrite semantics):
  - The interior (rows/cols 1..128 of the 130x130 grid) is OVERWRITTEN by a
    5-tap stencil that only involves xl = x[:, -1] (last input channel):
      out[m,o,1+h,1+w] = xl[m,h,w]
                       + K(1,0) xl[m,h,w+1] + K(0,2) xl[m,h+1,w-1]
                       + K(0,1) xl[m,h+1,w] + K(0,0) xl[m,h+1,w+1]
    where K(a,b) = kron(Gl[o,a,b], Gl[o,a,b]) acting on the 3x3 block as a
    9-vector, Gl = G[:, -1].
  - Only the 1-pixel border frame keeps the full accumulation over all input
    channels; each border line is a 3-tap 1-D conv with 9x9 channel mixes.

Sharding: pure data parallel over batch (2 images per core). G / weights are
built on host (tiny) and replicated. Per core the device does:
  - interior: out[144, n] = W[45,144]^T @ xin[45, n] over n = padded spatial
    positions; xin rows = 5 shifted copies of the 9-channel input, loaded by
    5 DMAs at column offsets {0, 1, 128, 129, 130} from a zero-padded
    [9, (H+1)*(W+1)] layout (pad row/col make all shifts safe).
  - border: 4 lines, each K=432 contraction split into 4 matmuls of K=108
    accumulated in PSUM.

Perf structure: float32r matmuls (1 cyc/col at N>=256), DMAs batched into
4096-position spans and issued from three engines (gpsimd: loads,
sync/scalar: stores) to spread HWDGE queue issue cost.
"""

import numpy as np

# ---------------- problem constants (hardcoded per contract) ----------------
B, CIN, O, H, W = 16, 16, 16, 128, 128
NCORES = 8
BLOC = B // NCORES          # 2 images per core
S = W + 1                   # padded row stride (zero pad col at w=W)
PH = H + 1                  # padded rows per image (zero pad row at h=H)
IMG = PH * S                # 16641 padded positions per image
NPOS = BLOC * IMG           # 33282
TILE_N = 512                # positions per matmul
NT = -(-NPOS // TILE_N)     # 66 matmul tiles
NCOLS_OUT = NT * TILE_N     # 33792
XIN_COLS = NCOLS_OUT + 132  # tail pad so +130 offset loads stay in bounds
TAP_OFFS = (0, 1, S - 1, S, S + 1)
SPAN = 4096                 # positions per DMA span (8 matmul tiles)
NSPAN = -(-NCOLS_OUT // SPAN)
N_TB = BLOC * (W + 2)       # 260 (top/bottom line positions)
N_LR = BLOC * H             # 256 (left/right line positions)
BOR_OFF = (0, N_TB, 2 * N_TB, 2 * N_TB + N_LR)
BOR_COLS = 2 * N_TB + 2 * N_LR  # 1032
KBOR = CIN * 27             # 432 contraction rows for border
IGRPS = 4
KG = KBOR // IGRPS          # 108
EPS = 1e-7

_CACHE = {}


# ---------------- host-side math (tiny) ----------------
def _build_G(g):
    # g: [O, I, 8, 3] f32 -> Cayley matrices G [O, I, 3, 3, 3, 3]
    idx = np.array([[0, 1, 2], [3, 4, 5], [6, 7, 4]])
    gk = g[:, :, idx, :]
    a_, b_, c_ = gk[..., 0], gk[..., 1], gk[..., 2]
    z = np.zeros_like(a_)
    A = np.stack([
        np.stack([z, a_, b_], -1),
        np.stack([-a_, z, c_], -1),
        np.stack([-b_, -c_, z], -1),
    ], -2)
    I3 = np.eye(3, dtype=g.dtype)
    Xm = A.copy(); Xm[:, :, 1, 1] = I3
    Idm = (I3 - A); Idm[:, :, 1, 1] = I3
    bm = Idm
    b00 = bm[..., 0, 0]; b01 = bm[..., 0, 1]; b02 = bm[..., 0, 2]
    b10 = bm[..., 1, 0]; b11 = bm[..., 1, 1]; b12 = bm[..., 1, 2]
    b20 = bm[..., 2, 0]; b21 = bm[..., 2, 1]; b22 = bm[..., 2, 2]
    det = (b00 * (b11 * b22 - b12 * b21)
           - b01 * (b10 * b22 - b12 * b20)
           + b02 * (b10 * b21 - b11 * b20))
    cof = np.stack([
        np.stack([b11 * b22 - b12 * b21, b02 * b21 - b01 * b22, b01 * b12 - b02 * b11], -1),
        np.stack([b12 * b20 - b10 * b22, b00 * b22 - b02 * b20, b02 * b10 - b00 * b12], -1),
        np.stack([b10 * b21 - b11 * b20, b01 * b20 - b00 * b21, b00 * b11 - b01 * b10], -1),
    ], -2)
    inv = cof / (det + EPS)[..., None, None]
    return (inv @ (I3 + Xm)).astype(np.float32)


def _weights(g):
    """Returns (w_int [45, 144], w_bor [108, 2304]) matmul lhsT weights."""
    G = _build_G(g)                                     # [O, I, 3, 3, 3, 3]
    K9 = np.einsum('oiabpq,oiabts->oiabptqs', G, G).reshape(O, CIN, 3, 3, 9, 9)
    K9 = K9.astype(np.float32)

    # interior: taps in DMA-offset order [0, +1, +S-1, +S, +S+1]
    K_int = np.empty((5, O, 9, 9), np.float32)
    K_int[0] = np.eye(9, dtype=np.float32)
    K_int[1] = K9[:, CIN - 1, 1, 0]
    K_int[2] = K9[:, CIN - 1, 0, 2]
    K_int[3] = K9[:, CIN - 1, 0, 1]
    K_int[4] = K9[:, CIN - 1, 0, 0]
    # w_int[t*9+q, o*9+p] = K_int[t, o, p, q]
    w_int = np.ascontiguousarray(K_int.transpose(0, 3, 1, 2).reshape(45, O * 9))

    # border lines: top (a=0, taps b), bottom (a=2, taps b),
    #               left (b=0, taps a), right (b=2, taps a)
    w_bor = np.empty((KG, 4 * IGRPS * O * 9), np.float32)
    sels = [K9[:, :, 0, :], K9[:, :, 2, :], K9[:, :, :, 0], K9[:, :, :, 2]]
    for L, KL in enumerate(sels):                       # KL [O, I, 3, 9p, 9q]
        WL = KL.transpose(1, 2, 4, 0, 3).reshape(KBOR, O * 9)  # rows (i,t,q)
        for j in range(IGRPS):
            w_bor[:, (L * IGRPS + j) * 144:(L * IGRPS + j + 1) * 144] = \
                WL[j * KG:(j + 1) * KG]
    return w_int, w_bor


def _prep_xin_int(x, c):
    xsl = x[BLOC * c:BLOC * c + BLOC, CIN - 1]          # [2, H, W, 3, 3]
    xl9 = xsl.reshape(BLOC, H, W, 9).transpose(3, 0, 1, 2)
    tmp = np.zeros((9, BLOC, PH, S), np.float32)
    tmp[:, :, :H, :W] = xl9
    out = np.zeros((9, XIN_COLS), np.float32)
    out[:, :NPOS] = tmp.reshape(9, NPOS)
    return out


def _prep_xin_bor(x, c):
    x9 = x[BLOC * c:BLOC * c + BLOC].reshape(BLOC, CIN, H, W, 9)
    bor = np.zeros((KBOR, BOR_COLS), np.float32)
    for li, h_in in ((0, 0), (1, H - 1)):               # top, bottom
        rT = x9[:, :, h_in].transpose(1, 3, 0, 2)       # [I, 9, 2, W]
        blk = np.zeros((CIN, 3, 9, BLOC, W + 2), np.float32)
        for b in range(3):
            blk[:, b, :, :, b:b + W] = rT
        bor[:, BOR_OFF[li]:BOR_OFF[li] + N_TB] = blk.reshape(KBOR, N_TB)
    for li, w_in in ((2, 0), (3, W - 1)):               # left, right
        cT = x9[:, :, :, w_in].transpose(1, 3, 0, 2)    # [I, 9, 2, H]
        blk = np.zeros((CIN, 3, 9, BLOC, H), np.float32)
        for a in range(3):
            h_lo = max(0, 1 - a)
            h_hi = min(H - 1, H - a)
            blk[:, a, :, :, h_lo + a - 1:h_hi + a] = cT[:, :, :, h_lo:h_hi + 1]
        bor[:, BOR_OFF[li]:BOR_OFF[li] + N_LR] = blk.reshape(KBOR, N_LR)
    return bor


def _assemble(results):
    out = np.empty((B, O, H + 2, W + 2, 3, 3), np.float32)
    for c in range(NCORES):
        ms = slice(BLOC * c, BLOC * c + BLOC)
        oi = results[c]["out_int"]
        t = oi[:, :NPOS].reshape(O, 9, BLOC, PH, S)[:, :, :, :H, :W]
        out[ms, :, 1:H + 1, 1:W + 1] = \
            t.transpose(2, 0, 3, 4, 1).reshape(BLOC, O, H, W, 3, 3)
        ob = results[c]["out_bor"].reshape(O, 9, BOR_COLS)
        top = ob[:, :, 0:N_TB].reshape(O, 9, BLOC, W + 2)
        out[ms, :, 0, :] = top.transpose(2, 0, 3, 1).reshape(BLOC, O, W + 2, 3, 3)
        bot = ob[:, :, N_TB:2 * N_TB].reshape(O, 9, BLOC, W + 2)
        out[ms, :, H + 1, :] = bot.transpose(2, 0, 3, 1).reshape(BLOC, O, W + 2, 3, 3)
        lef = ob[:, :, BOR_OFF[2]:BOR_OFF[2] + N_LR].reshape(O, 9, BLOC, H)
        out[ms, :, 1:H + 1, 0] = lef.transpose(2, 0, 3, 1).reshape(BLOC, O, H, 3, 3)
        rig = ob[:, :, BOR_OFF[3]:BOR_OFF[3] + N_LR].reshape(O, 9, BLOC, H)
        out[ms, :, 1:H + 1, W + 1] = rig.transpose(2, 0, 3, 1).reshape(BLOC, O, H, 3, 3)
    return out


# ---------------- device kernel ----------------
def _build_module():
    if "nc" in _CACHE:
        return _CACHE["nc"]
    import concourse.bass as bass
    import concourse.mybir as mybir
    import concourse.tile as tile
    from concourse import bacc

    f32 = mybir.dt.float32
    f32r = mybir.dt.float32r
    nc = bacc.Bacc(None, target_bir_lowering=False)

    xin_int = nc.dram_tensor("xin_int", [9, XIN_COLS], f32r, kind="ExternalInput")
    xin_bor = nc.dram_tensor("xin_bor", [KBOR, BOR_COLS], f32r, kind="ExternalInput")
    w_int_d = nc.dram_tensor("w_int", [45, O * 9], f32r, kind="ExternalInput")
    w_bor_d = nc.dram_tensor("w_bor", [KG, 4 * IGRPS * O * 9], f32r, kind="ExternalInput")
    out_int = nc.dram_tensor("out_int", [O * 9, NCOLS_OUT], f32, kind="ExternalOutput")
    out_bor = nc.dram_tensor("out_bor", [O * 9, BOR_COLS], f32, kind="ExternalOutput")

    with tile.TileContext(nc) as tc:
        with (
            tc.tile_pool(name="const", bufs=1) as constp,
            tc.tile_pool(name="borin", bufs=1) as borp,
            tc.tile_pool(name="sin", bufs=2) as inp,
            tc.tile_pool(name="sout", bufs=2) as outp,
            tc.tile_pool(name="ps", bufs=2, space=bass.MemorySpace.PSUM) as psp,
        ):
            w_int_sb = constp.tile([45, O * 9], f32r, tag="wint")
            nc.gpsimd.dma_start(w_int_sb[:], w_int_d[:])
            w_bor_sb = constp.tile([KG, 4 * IGRPS * O * 9], f32r, tag="wbor")
            nc.gpsimd.dma_start(w_bor_sb[:], w_bor_d[:])
            bor_in = []
            for j in range(IGRPS):
                t = borp.tile([KG, BOR_COLS], f32r, tag=f"bx{j}")
                nc.gpsimd.dma_start(t[:], xin_bor[j * KG:(j + 1) * KG, :])
                bor_in.append(t)
            wiA = w_int_sb[:, 0:72]
            wiB = w_int_sb[:, 72:144]

            for sp in range(NSPAN):
                s0 = sp * SPAN
                ncols = min(SPAN, NCOLS_OUT - s0)
                sb = inp.tile([45, SPAN], f32r, tag="xin")
                # two grouped tap loads with a replication outer dim:
                # taps {0,+1} -> partitions 0..17, taps {+128,+129,+130} ->
                # partitions 18..44 (alternating HWDGE issue engines)
                eng = nc.sync if (sp % 2 == 0) else nc.scalar
                eng.dma_start(
                    sb[0:18, :ncols],
                    bass.AP(xin_int, s0, [[1, 2], [XIN_COLS, 9], [1, ncols]]),
                )
                eng.dma_start(
                    sb[18:45, :ncols],
                    bass.AP(xin_int, s0 + S - 1,
                            [[1, 3], [XIN_COLS, 9], [1, ncols]]),
                )
                oa = outp.tile([72, SPAN], f32, tag="oa")
                ob = outp.tile([72, SPAN], f32, tag="ob")
                for jj in range(0, ncols, 2 * TILE_N):
                    nj = min(2 * TILE_N, ncols - jj)
                    pa = psp.tile([72, 2 * TILE_N], f32, tag="pa")
                    pb = psp.tile([72, 2 * TILE_N], f32, tag="pb")
                    for q0 in range(0, nj, TILE_N):
                        rhs = sb[:, jj + q0:jj + q0 + TILE_N]
                        nc.tensor.matmul(pa[:, q0:q0 + TILE_N], wiA, rhs,
                                         start=True, stop=True)
                    for q0 in range(0, nj, TILE_N):
                        rhs = sb[:, jj + q0:jj + q0 + TILE_N]
                        nc.tensor.matmul(pb[:, q0:q0 + TILE_N], wiB, rhs,
                                         start=True, stop=True)
                    nc.vector.tensor_copy(oa[:, jj:jj + nj], pa[:, :nj])
                    nc.vector.tensor_copy(ob[:, jj:jj + nj], pb[:, :nj])
                nc.sync.dma_start(out_int[0:72, s0:s0 + ncols], oa[:, :ncols])
                nc.scalar.dma_start(out_int[72:144, s0:s0 + ncols], ob[:, :ncols])

            for L in range(4):
                ncol = N_TB if L < 2 else N_LR
                coff = BOR_OFF[L]
                for half in range(2):
                    ps = psp.tile([72, 2 * TILE_N], f32, tag="pa")
                    for j in range(IGRPS):
                        wc = (L * IGRPS + j) * 144 + half * 72
                        nc.tensor.matmul(
                            ps[:, :ncol],
                            w_bor_sb[:, wc:wc + 72],
                            bor_in[j][:, coff:coff + ncol],
                            start=(j == 0),
                            stop=(j == IGRPS - 1),
                        )
                    obr = outp.tile([72, N_TB], f32, tag="obor")
                    nc.vector.tensor_copy(obr[:, :ncol], ps[:, :ncol])
                    nc.sync.dma_start(
                        out_bor[half * 72:half * 72 + 72, coff:coff + ncol],
                        obr[:, :ncol],
                    )

    nc.finalize()
    _CACHE["nc"] = nc
    return nc


def kernel(x, g):
    x = np.ascontiguousarray(np.asarray(x, dtype=np.float32))
    g = np.asarray(g, dtype=np.float32)
    w_int, w_bor = _weights(g)
    in_maps = []
    for c in range(NCORES):
        in_maps.append({
            "xin_int": _prep_xin_int(x, c),
            "xin_bor": _prep_xin_bor(x, c),
            "w_int": w_int,
            "w_bor": w_bor,
        })
    nc = _build_module()
    from concourse.bass_utils import run_bass_kernel_spmd
    res = run_bass_kernel_spmd(nc, in_maps, list(range(NCORES)))
    _CACHE["last_result"] = res
    return _assemble(res.results)
